# revision 1
# baseline (speedup 1.0000x reference)
"""Per-image piecewise-linear LUT (histogram binning) kernel for Trainium2.

Strategy (pure data-parallel over 8 NeuronCores, batch sharded 2 per core):
- Host precomputes the tiny normalized LUT per (b, c): y = rescaled cumsum of
  softplus(un_normalized_y), then packs (y0[j], dy[j]) as two fp16 in one u32.
- On-device per core: 6 images of [128 partitions x 8192]. Per chunk:
    u    = u32(x * 64 - 0.5)        (DVE; fp32->u32 convert is round-nearest
                                     => exact floor for x in [0, 1))
    e    = pooltable[u]             (GPSIMD PoolBufferLoad+Gather: a true
                                     per-partition, per-element 64-entry LUT)
    frac = x * 64 - f32(u)          (custom DVE op; read FIFO converts u32)
    out  = lo16(e) + frac * hi16(e) (DVE, fp16 pair views)
- The raw Gather/PoolBufferLoad ISA instructions cannot carry semaphores
  (walrus rejects sync on unknown structs); drains bracket them and all
  cross-engine syncs land on the drains / are wired manually.
"""

import sys

sys.path.insert(0, "/opt/trn_rl_repo")

import numpy as np

B, C, H, W = 16, 3, 1024, 1024
K = 64
NCORES = 8
BPC = B // NCORES  # batches per core
IMGS = BPC * C  # images per core
P = 128
FREE = H * W // P  # 8192
CHUNK = 4096
NCHUNK = FREE // CHUNK
TBL = 128  # pool buffer entries (pow2 >= 65)

_cached = {}


def _get_frac_op():
    """out = in0 * s0 - f32(in1); registered once as a custom DVE op."""
    from concourse import dve_ops
    from concourse.dve_spec import Spec, Src0, Src1, C0, lower
    from concourse.dve_uop import DveOpSpec

    for op in dve_ops.OPS:
        if op.name == "ANT_FRAC_SCALE":
            return op
    spec = Spec(
        body=Src0 * C0 - Src1,
        reference=lambda in0, in1, s0, s1, imm2: in0 * s0 - in1,
    )
    opcode = dve_ops._CUSTOM_DVE_ROW_BASE + len(dve_ops.OPS)
    sha = {}
    for ver in ("v3", "v4"):
        s = DveOpSpec(
            name="ANT_FRAC_SCALE",
            opcode=opcode,
            uops=lower(spec, ver=ver),
            rd1_en=True,
        )
        sha[ver] = s.sha(ver)
    op = dve_ops.DveOp("ANT_FRAC_SCALE", spec, subdim=False, uops_sha=sha)
    dve_ops.OPS.append(op)
    dve_ops._SUB_OPCODE_FOR_NAME[op.name] = opcode
    dve_ops.CUSTOM_DVE_SPECS[op.name] = spec
    return op


def _build(loop_n=None, mode="full"):
    import contextlib
    import concourse.mybir as mybir
    from concourse.bacc import Bacc
    from concourse.tile import TileContext
    from concourse.tile_rust import add_dep_helper
    import concourse.bass_interp as _bi

    # Tile's scheduling simulator doesn't know these opcodes; no-op them there.
    _orig_visit = _bi._visit_InstISA

    def _patched_visit(isa, instruction, core_sim):
        if instruction.isa_opcode in (
            isa.Opcode.NEURON_ISA_TPB_OPCODE_POOL_BUFFER_LOAD.value,
            isa.Opcode.NEURON_ISA_TPB_OPCODE_GATHER.value,
        ):
            return
        return _orig_visit(isa, instruction, core_sim)

    _bi._visit_InstISA = _patched_visit

    frac_op = _get_frac_op()

    nc = Bacc()
    dt = nc.isa.get_enum("NEURON_ISA_TPB_DTYPE")
    Op = nc.isa.Opcode
    ALU = mybir.AluOpType

    xs_d = nc.dram_tensor("xs", [IMGS, H, W], mybir.dt.float32, kind="ExternalInput")
    tb_d = nc.dram_tensor("tb", [IMGS, P, TBL], mybir.dt.uint32, kind="ExternalInput")
    os_d = nc.dram_tensor("os", [IMGS, H, W], mybir.dt.float32, kind="ExternalOutput")

    xs_r = xs_d[:].rearrange("i (p r) c -> i p (r c)", p=P)
    os_r = os_d[:].rearrange("i (p r) c -> i p (r c)", p=P)

    NB = 4  # buffer depth

    with (
        nc.sbuf_tensor("tbl_all", [P, IMGS * TBL], mybir.dt.uint32) as tbl_all,
        nc.sbuf_tensor("tbl_cp", [P, IMGS * TBL], mybir.dt.uint32) as tbl_cp,
        nc.sbuf_tensor("xb", [P, NB * CHUNK], mybir.dt.float32) as xb,
        nc.sbuf_tensor("ub", [P, NB * CHUNK], mybir.dt.uint32) as ub,
        nc.sbuf_tensor("eb", [P, NB * CHUNK], mybir.dt.uint32) as eb,
        TileContext(nc) as tc,
    ):
        ub_off, _ = nc.gpsimd._ap_to_byte_offset(ub[:])
        eb_off, _ = nc.gpsimd._ap_to_byte_offset(eb[:])
        tcp_off, _ = nc.gpsimd._ap_to_byte_offset(tbl_cp[:])
        U32 = dt.NEURON_ISA_TPB_DTYPE_UINT32.value

        loop_cm = (
            tc.For_i(0, loop_n, 1) if loop_n is not None else contextlib.nullcontext()
        )
        with loop_cm:
            # table load + a DVE copy so pool's wait collapses onto the DVE clock
            for img in range(IMGS):
                nc.sync.dma_start(tbl_all[:, img * TBL : (img + 1) * TBL], tb_d[img])
            tbl_touch = nc.vector.tensor_copy(tbl_cp[:], tbl_all[:])

            hist = {}  # slot -> last DVE reader of the e-buffer in that slot
            prev_post = None
            pend = None
            k = 0

            def _emit_interp(p):
                e16 = p["e_t"].bitcast(mybir.dt.float16).rearrange(
                    "p (n two) -> p n two", two=2
                )
                tt_m = nc.vector.tensor_tensor(
                    p["m_t"], p["fr_t"], e16[:, :, 1], ALU.mult
                )
                add_dep_helper(tt_m.ins, p["post"].ins, sync=True, reason="g done")
                tt_o = nc.vector.tensor_tensor(
                    p["o_t"], p["m_t"], e16[:, :, 0], ALU.add
                )
                add_dep_helper(tt_o.ins, p["post"].ins, sync=True, reason="g done")
                nc.sync.dma_start(
                    os_r[p["img"], :, p["f0"] : p["f0"] + CHUNK], p["o_t"]
                )
                hist[p["slot"]] = tt_o

            for img in range(IMGS):
                for cidx in range(NCHUNK):
                    slot = k % NB
                    f0 = cidx * CHUNK
                    so = slot * CHUNK  # sbuf column offset
                    x_t = xb[:, so : so + CHUNK]
                    u_t = ub[:, so : so + CHUNK]
                    fr_t = x_t  # frac in place over x
                    e_t = eb[:, so : so + CHUNK]
                    # m aliases x (x dead after u/frac); o aliases u (dead after
                    # gather+frac; tt_o already syncs on the post-gather drain)
                    m_t = x_t
                    o_t = u_t.bitcast(mybir.dt.float32)

                    nc.sync.dma_start(x_t, xs_r[img, :, f0 : f0 + CHUNK])

                    # u = u32(64x - 0.5); frac = 64x - f32(u)
                    ts_u = nc.vector.tensor_scalar(
                        u_t, x_t, 64.0, 0.5, ALU.mult, ALU.subtract
                    )
                    nc.vector._custom_dve(
                        frac_op, out=fr_t, in0=x_t, in1=u_t, s0=64.0
                    )

                    # pool: single drain per chunk — serves as the previous
                    # gather's completion fence AND this gather's input wait
                    pre = nc.gpsimd.drain()
                    if prev_post is not None:
                        add_dep_helper(
                            pre.ins, prev_post.ins, sync=False, reason="pool order"
                        )
                    add_dep_helper(pre.ins, ts_u.ins, sync=True, reason="u ready")
                    if k >= NB:
                        add_dep_helper(
                            pre.ins, hist[slot].ins, sync=True, reason="e WAR"
                        )
                    if cidx == 0:
                        if img == 0:
                            add_dep_helper(
                                pre.ins, tbl_touch.ins, sync=True, reason="tables"
                            )
                        pbl = nc.gpsimd.isa(
                            Op.NEURON_ISA_TPB_OPCODE_POOL_BUFFER_LOAD,
                            {
                                "src_mem_pattern": {
                                    "start_addr": {
                                        "addr_immediate": int(tcp_off) + img * TBL * 4
                                    },
                                    "num_elem": [TBL, 1, 1, 1],
                                    "step_elem": [1, 0, 0, 0],
                                },
                                "in_dtype": U32,
                                "num_active_channels": P,
                                "start_index": 0,
                                "mask": TBL - 1,
                            },
                        )
                        add_dep_helper(pbl.ins, pre.ins, sync=False, reason="pool order")
                        gdep = pbl
                    else:
                        gdep = pre
                    gt = nc.gpsimd.isa(
                        Op.NEURON_ISA_TPB_OPCODE_GATHER,
                        {
                            "src_mem_pattern": {
                                "start_addr": {"addr_immediate": int(ub_off) + so * 4},
                                "num_elem": [CHUNK, 1, 1, 1],
                                "step_elem": [1, 0, 0, 0],
                            },
                            "dst_mem_pattern": {
                                "start_addr": {"addr_immediate": int(eb_off) + so * 4},
                                "num_elem": [CHUNK, 1, 1, 1],
                                "step_elem": [1, 0, 0, 0],
                            },
                            "in_dtype": U32,
                            "out_dtype": U32,
                            "num_active_channels": P,
                            "index_miss_behavior": 0,
                            "immediate": {"imm_bitvec_uint32": 0},
                            "free_pool_buffer": 0,
                        },
                    )
                    add_dep_helper(gt.ins, gdep.ins, sync=False, reason="pool order")

                    # interpolate the PREVIOUS chunk now; `pre` (emitted after
                    # the previous gather in pool order) is its fence
                    if pend is not None:
                        pend["post"] = pre
                        _emit_interp(pend)
                    pend = dict(
                        slot=slot, fr_t=fr_t, e_t=e_t, m_t=m_t, o_t=o_t,
                        post=None, img=img, f0=f0,
                    )
                    prev_post = gt
                    k += 1
            if pend is not None:
                fin = nc.gpsimd.drain()
                add_dep_helper(fin.ins, prev_post.ins, sync=False, reason="pool order")
                pend["post"] = fin
                _emit_interp(pend)

    nc.finalize()
    return nc


def _tables(un_normalized_y: np.ndarray) -> np.ndarray:
    """[B, C, TBL] u32 packed (fp16 y0 | fp16 dy << 16)."""
    u = un_normalized_y.astype(np.float64)
    h = np.logaddexp(0.0, u)  # softplus
    y = np.cumsum(h, axis=2)
    y0 = y[:, :, :1]
    yn = y[:, :, -1:]
    y = ((y - y0) / (yn - y0)).astype(np.float32)  # [B, C, K+1], y[0]=0, y[K]=1
    a = y[:, :, :K]
    d = y[:, :, 1:] - y[:, :, :K]
    a16 = a.astype(np.float16).view(np.uint16).astype(np.uint32)
    d16 = d.astype(np.float16).view(np.uint16).astype(np.uint32)
    pk = a16 | (d16 << 16)  # [B, C, K]
    out = np.zeros((B, C, TBL), dtype=np.uint32)
    out[:, :, :K] = pk
    one16 = np.float16(1.0).view(np.uint16).astype(np.uint32)
    out[:, :, K] = one16
    return out


def _in_maps(x: np.ndarray, uy: np.ndarray):
    pk = _tables(uy)
    in_maps = []
    for c in range(NCORES):
        xs = x[c * BPC : (c + 1) * BPC].reshape(IMGS, H, W)
        tb = np.ascontiguousarray(
            np.broadcast_to(
                pk[c * BPC : (c + 1) * BPC].reshape(IMGS, 1, TBL), (IMGS, P, TBL)
            )
        )
        in_maps.append({"xs": np.ascontiguousarray(xs), "tb": tb})
    return in_maps


def kernel(x: np.ndarray, un_normalized_y: np.ndarray) -> np.ndarray:
    from concourse import bass_utils

    x = np.ascontiguousarray(np.asarray(x, dtype=np.float32))
    uy = np.asarray(un_normalized_y, dtype=np.float32)

    if "nc" not in _cached:
        _cached["nc"] = _build()
    nc = _cached["nc"]

    res = bass_utils.run_bass_kernel_spmd(
        nc, _in_maps(x, uy), core_ids=list(range(NCORES))
    )
    out = np.empty((B, C, H, W), dtype=np.float32)
    for c in range(NCORES):
        out[c * BPC : (c + 1) * BPC] = res.results[c]["os"].reshape(BPC, C, H, W)
    return out



# revision 3
# speedup vs baseline: 1.8627x; 1.8627x over previous
"""Per-image piecewise-linear LUT (histogram binning) kernel for Trainium2.

Strategy (pure data-parallel over 8 NeuronCores, batch sharded 2 per core):
- Host precomputes, per (b, c), a dense 512-entry nearest-neighbor table
  sampling the normalized curve at bin midpoints: tbl[j] = y((j+0.5)/S),
  S = 511.5.  With 512 bins the midpoint-sampling error is ~1e-3 norm-rel,
  far inside the 2e-2 gate, and it removes the on-device interpolation
  entirely.
- x ships as fp16 (halves input HBM traffic; fp16 quantization of x only
  perturbs the bin index by <0.3 bins).  Output is written fp16 and
  upcast on host.
- On-device per core: 6 images of [128 partitions x 8192 fp16].  Per image:
    u16 idx = u16(x * 511.5 - 0.5)   (one DVE tensor_scalar, 4x mode)
    out     = pooltable[idx]          (pool-engine PoolBufferLoad+Gather,
                                       512-entry per-channel table)
    DMA out (fp16)
- The raw Gather/PoolBufferLoad ISA instructions cannot carry semaphores
  (walrus rejects sync on unknown structs); drains bracket them and all
  cross-engine syncs land on the drains / are wired manually.
"""

import sys

sys.path.insert(0, "/opt/trn_rl_repo")

import numpy as np

B, C, H, W = 16, 3, 1024, 1024
K = 64
NCORES = 8
BPC = B // NCORES  # batches per core
IMGS = BPC * C  # images per core
P = 128
FREE = H * W // P  # 8192
CHUNK = 8192
NCHUNK = FREE // CHUNK
TBL = 512  # pool buffer entries (hardware max 512)
S = 511.5  # index scale: u = round_nearest(x*S - 0.5) in [0, 511] for x in [0,1]
NB = 3  # buffer depth

_cached = {}


def _build(loop_n=None):
    import contextlib
    import concourse.mybir as mybir
    from concourse.bacc import Bacc
    from concourse.tile import TileContext
    from concourse.tile_rust import add_dep_helper
    import concourse.bass_interp as _bi

    # Tile's scheduling simulator doesn't know these opcodes; no-op them there.
    _orig_visit = _bi._visit_InstISA

    def _patched_visit(isa, instruction, core_sim):
        if instruction.isa_opcode in (
            isa.Opcode.NEURON_ISA_TPB_OPCODE_POOL_BUFFER_LOAD.value,
            isa.Opcode.NEURON_ISA_TPB_OPCODE_GATHER.value,
        ):
            return
        return _orig_visit(isa, instruction, core_sim)

    _bi._visit_InstISA = _patched_visit

    nc = Bacc()
    dt = nc.isa.get_enum("NEURON_ISA_TPB_DTYPE")
    Op = nc.isa.Opcode
    ALU = mybir.AluOpType

    xs_d = nc.dram_tensor("xs", [IMGS, H, W], mybir.dt.float16, kind="ExternalInput")
    tb_d = nc.dram_tensor("tb", [IMGS, P, TBL], mybir.dt.float16, kind="ExternalInput")
    os_d = nc.dram_tensor("os", [IMGS, H, W], mybir.dt.float16, kind="ExternalOutput")

    xs_r = xs_d[:].rearrange("i (p r) c -> i p (r c)", p=P)
    os_r = os_d[:].rearrange("i (p r) c -> i p (r c)", p=P)

    with (
        nc.sbuf_tensor("tbl_all", [P, IMGS * TBL], mybir.dt.float16) as tbl_all,
        nc.sbuf_tensor("tbl_cp", [P, IMGS * TBL], mybir.dt.float16) as tbl_cp,
        nc.sbuf_tensor("xb", [P, NB * CHUNK], mybir.dt.float16) as xb,
        nc.sbuf_tensor("ub", [P, NB * CHUNK], mybir.dt.uint16) as ub,
        nc.sbuf_tensor("ob", [P, NB * CHUNK], mybir.dt.float16) as ob,
        TileContext(nc) as tc,
    ):
        ub_off, _ = nc.gpsimd._ap_to_byte_offset(ub[:])
        ob_off, _ = nc.gpsimd._ap_to_byte_offset(ob[:])
        tcp_off, _ = nc.gpsimd._ap_to_byte_offset(tbl_cp[:])
        U16 = dt.NEURON_ISA_TPB_DTYPE_UINT16.value
        F16 = dt.NEURON_ISA_TPB_DTYPE_FP16.value

        loop_cm = (
            tc.For_i(0, loop_n, 1) if loop_n is not None else contextlib.nullcontext()
        )
        with loop_cm:
            # table load + a DVE copy so pool's wait collapses onto the DVE clock
            for img in range(IMGS):
                nc.sync.dma_start(tbl_all[:, img * TBL : (img + 1) * TBL], tb_d[img])
            tbl_touch = nc.vector.tensor_copy(tbl_cp[:], tbl_all[:])

            fences = {}  # k -> drain emitted just after gather k-1 (pool order)
            outs = {}  # k -> output DMA instruction for chunk k
            pend = None  # (k, img, f0, slot) awaiting its post-gather fence
            prev_pool = None
            k = 0

            def _emit_out(p, fence):
                d = nc.sync.dma_start(
                    os_r[p["img"], :, p["f0"] : p["f0"] + CHUNK],
                    ob[:, p["slot"] * CHUNK : (p["slot"] + 1) * CHUNK],
                )
                add_dep_helper(d.ins, fence.ins, sync=True, reason="gather done")
                outs[p["k"]] = d

            for img in range(IMGS):
                for cidx in range(NCHUNK):
                    slot = k % NB
                    f0 = cidx * CHUNK
                    so = slot * CHUNK
                    x_t = xb[:, so : so + CHUNK]
                    u_t = ub[:, so : so + CHUNK]

                    nc.sync.dma_start(x_t, xs_r[img, :, f0 : f0 + CHUNK])

                    # idx = u16(S*x - 0.5): round-nearest fp32->u16 == floor(S*x)
                    ts_u = nc.vector.tensor_scalar(
                        u_t, x_t, float(S), 0.5, ALU.mult, ALU.subtract
                    )
                    if k >= NB:
                        # gather k-NB read this ub slot; its fence is fences[k-NB+1]
                        add_dep_helper(
                            ts_u.ins, fences[k - NB + 1].ins, sync=True,
                            reason="u WAR",
                        )

                    # pool: single drain per chunk — serves as the previous
                    # gather's completion fence AND this gather's input wait
                    pre = nc.gpsimd.drain()
                    fences[k] = pre
                    if prev_pool is not None:
                        add_dep_helper(
                            pre.ins, prev_pool.ins, sync=False, reason="pool order"
                        )
                    add_dep_helper(pre.ins, ts_u.ins, sync=True, reason="u ready")
                    if k >= NB:
                        # out-DMA k-NB still reads this ob slot
                        add_dep_helper(
                            pre.ins, outs[k - NB].ins, sync=True, reason="o WAR"
                        )
                    if cidx == 0:
                        if img == 0:
                            add_dep_helper(
                                pre.ins, tbl_touch.ins, sync=True, reason="tables"
                            )
                        pbl = nc.gpsimd.isa(
                            Op.NEURON_ISA_TPB_OPCODE_POOL_BUFFER_LOAD,
                            {
                                "src_mem_pattern": {
                                    "start_addr": {
                                        "addr_immediate": int(tcp_off) + img * TBL * 2
                                    },
                                    "num_elem": [TBL, 1, 1, 1],
                                    "step_elem": [1, 0, 0, 0],
                                },
                                "in_dtype": F16,
                                "num_active_channels": P,
                                "start_index": 0,
                                "mask": TBL - 1,
                            },
                        )
                        add_dep_helper(pbl.ins, pre.ins, sync=False, reason="pool order")
                        gdep = pbl
                    else:
                        gdep = pre
                    gt = nc.gpsimd.isa(
                        Op.NEURON_ISA_TPB_OPCODE_GATHER,
                        {
                            "src_mem_pattern": {
                                "start_addr": {"addr_immediate": int(ub_off) + so * 2},
                                "num_elem": [CHUNK, 1, 1, 1],
                                "step_elem": [1, 0, 0, 0],
                            },
                            "dst_mem_pattern": {
                                "start_addr": {"addr_immediate": int(ob_off) + so * 2},
                                "num_elem": [CHUNK, 1, 1, 1],
                                "step_elem": [1, 0, 0, 0],
                            },
                            "in_dtype": U16,
                            "out_dtype": F16,
                            "num_active_channels": P,
                            "index_miss_behavior": 0,
                            "immediate": {"imm_bitvec_uint32": 0},
                            "free_pool_buffer": 0,
                        },
                    )
                    add_dep_helper(gt.ins, gdep.ins, sync=False, reason="pool order")

                    # the drain just emitted fences the PREVIOUS gather; its
                    # output can ship now
                    if pend is not None:
                        _emit_out(pend, pre)
                    pend = dict(k=k, img=img, f0=f0, slot=slot)
                    prev_pool = gt
                    k += 1
            fin = nc.gpsimd.drain()
            add_dep_helper(fin.ins, prev_pool.ins, sync=False, reason="pool order")
            _emit_out(pend, fin)

    nc.finalize()
    return nc


def _tables(un_normalized_y: np.ndarray) -> np.ndarray:
    """[B, C, TBL] fp16: dense midpoint-sampled LUT of the normalized curve."""
    u = un_normalized_y.astype(np.float64)
    h = np.logaddexp(0.0, u)  # softplus
    y = np.cumsum(h, axis=2)
    y0 = y[:, :, :1]
    yn = y[:, :, -1:]
    y = (y - y0) / (yn - y0)  # [B, C, K+1], y[0]=0, y[K]=1

    t = np.minimum((np.arange(TBL, dtype=np.float64) + 0.5) / S, 1.0)  # midpoints
    scaled = t * K
    idx0 = np.clip(np.floor(scaled), 0, K - 1).astype(np.int64)
    alpha = scaled - idx0
    y_lo = y[:, :, idx0]  # [B, C, TBL]
    y_hi = y[:, :, idx0 + 1]
    val = y_lo + alpha * (y_hi - y_lo)
    return val.astype(np.float16)


def _in_maps(x: np.ndarray, uy: np.ndarray):
    pk = _tables(uy)
    x16 = x.astype(np.float16)
    in_maps = []
    for c in range(NCORES):
        xs = x16[c * BPC : (c + 1) * BPC].reshape(IMGS, H, W)
        tb = np.ascontiguousarray(
            np.broadcast_to(
                pk[c * BPC : (c + 1) * BPC].reshape(IMGS, 1, TBL), (IMGS, P, TBL)
            )
        )
        in_maps.append({"xs": np.ascontiguousarray(xs), "tb": tb})
    return in_maps


def kernel(x: np.ndarray, un_normalized_y: np.ndarray) -> np.ndarray:
    from concourse import bass_utils

    x = np.asarray(x, dtype=np.float32)
    uy = np.asarray(un_normalized_y, dtype=np.float32)

    if "nc" not in _cached:
        _cached["nc"] = _build()
    nc = _cached["nc"]

    res = bass_utils.run_bass_kernel_spmd(
        nc, _in_maps(x, uy), core_ids=list(range(NCORES))
    )
    out = np.empty((B, C, H, W), dtype=np.float32)
    for c in range(NCORES):
        out[c * BPC : (c + 1) * BPC] = (
            res.results[c]["os"].astype(np.float32).reshape(BPC, C, H, W)
        )
    return out


# revision 6
# speedup vs baseline: 1.8853x; 1.0122x over previous
"""Per-image piecewise-linear LUT (histogram binning) kernel for Trainium2.

Strategy (pure data-parallel over 8 NeuronCores, batch sharded 2 per core):
- Host precomputes, per (b, c), a dense 512-entry nearest-neighbor table
  sampling the normalized curve at bin midpoints: tbl[j] = y((j+0.5)/S),
  S = 511.5.  With 512 bins the midpoint-sampling error is ~1e-3 norm-rel,
  far inside the 2e-2 gate, and it removes the on-device interpolation
  entirely.
- x ships as fp16 (halves input HBM traffic; fp16 quantization of x only
  perturbs the bin index by <0.3 bins).  Output is written fp16 and
  upcast on host.
- On-device per core: 6 images of [128 partitions x 8192 fp16].  Per image:
    u16 idx = u16(x * 511.5 - 0.5)   (one DVE tensor_scalar, 4x mode)
    out     = pooltable[idx]          (pool-engine PoolBufferLoad+Gather,
                                       512-entry per-channel table)
    DMA out (fp16)
- The raw Gather/PoolBufferLoad ISA instructions cannot carry semaphores
  (walrus rejects sync on unknown structs); drains bracket them and all
  cross-engine syncs land on the drains / are wired manually.
"""

import sys

sys.path.insert(0, "/opt/trn_rl_repo")

import numpy as np

B, C, H, W = 16, 3, 1024, 1024
K = 64
NCORES = 8
BPC = B // NCORES  # batches per core
IMGS = BPC * C  # images per core
P = 128
FREE = H * W // P  # 8192
CHUNK = 8192
NCHUNK = FREE // CHUNK
TBL = 512  # pool buffer entries (hardware max 512)
S = 511.5  # index scale: u = round_nearest(x*S - 0.5) in [0, 511] for x in [0,1]
NB = 3  # buffer depth

_cached = {}


def _build(loop_n=None, mode="full"):
    import contextlib
    import concourse.mybir as mybir
    from concourse.bacc import Bacc
    from concourse.tile import TileContext
    from concourse.tile_rust import add_dep_helper
    import concourse.bass_interp as _bi

    # Tile's scheduling simulator doesn't know these opcodes; no-op them there.
    _orig_visit = _bi._visit_InstISA

    def _patched_visit(isa, instruction, core_sim):
        if instruction.isa_opcode in (
            isa.Opcode.NEURON_ISA_TPB_OPCODE_POOL_BUFFER_LOAD.value,
            isa.Opcode.NEURON_ISA_TPB_OPCODE_GATHER.value,
        ):
            return
        return _orig_visit(isa, instruction, core_sim)

    _bi._visit_InstISA = _patched_visit

    nc = Bacc()
    dt = nc.isa.get_enum("NEURON_ISA_TPB_DTYPE")
    Op = nc.isa.Opcode
    ALU = mybir.AluOpType

    xs_d = nc.dram_tensor("xs", [IMGS, H, W], mybir.dt.float16, kind="ExternalInput")
    tb_d = nc.dram_tensor("tb", [IMGS, P, TBL], mybir.dt.float16, kind="ExternalInput")
    os_d = nc.dram_tensor("os", [IMGS, H, W], mybir.dt.float16, kind="ExternalOutput")

    xs_r = xs_d[:].rearrange("i (p r) c -> i p (r c)", p=P)
    os_r = os_d[:].rearrange("i (p r) c -> i p (r c)", p=P)

    with (
        nc.sbuf_tensor("tbl_all", [P, IMGS * TBL], mybir.dt.float16) as tbl_all,
        nc.sbuf_tensor("tbl_cp", [P, IMGS * TBL], mybir.dt.float16) as tbl_cp,
        nc.sbuf_tensor("xb", [P, NB * CHUNK], mybir.dt.float16) as xb,
        nc.sbuf_tensor("ub", [P, NB * CHUNK], mybir.dt.uint16) as ub,
        nc.sbuf_tensor("ob", [P, NB * CHUNK], mybir.dt.float16) as ob,
        TileContext(nc) as tc,
    ):
        ub_off, _ = nc.gpsimd._ap_to_byte_offset(ub[:])
        ob_off, _ = nc.gpsimd._ap_to_byte_offset(ob[:])
        tcp_off, _ = nc.gpsimd._ap_to_byte_offset(tbl_cp[:])
        U16 = dt.NEURON_ISA_TPB_DTYPE_UINT16.value
        F16 = dt.NEURON_ISA_TPB_DTYPE_FP16.value

        loop_cm = (
            tc.For_i(0, loop_n, 1) if loop_n is not None else contextlib.nullcontext()
        )
        if mode == "dma":
            with loop_cm:
                for img in range(IMGS):
                    for cidx in range(NCHUNK):
                        k = img * NCHUNK + cidx
                        slot = k % NB
                        f0 = cidx * CHUNK
                        so = slot * CHUNK
                        nc.sync.dma_start(
                            xb[:, so : so + CHUNK], xs_r[img, :, f0 : f0 + CHUNK]
                        )
                        nc.scalar.dma_start(
                            os_r[img, :, f0 : f0 + CHUNK], ob[:, so : so + CHUNK]
                        )
            nc.finalize()
            return nc
        if mode == "pool":
            with loop_cm:
                for img in range(IMGS):
                    nc.sync.dma_start(
                        tbl_all[:, img * TBL : (img + 1) * TBL], tb_d[img]
                    )
                tbl_touch = nc.vector.tensor_copy(tbl_cp[:], tbl_all[:])
                zed = nc.vector.memset(ub[:], 0)
                prev_pool = None
                for img in range(IMGS):
                    for cidx in range(NCHUNK):
                        k = img * NCHUNK + cidx
                        so = (k % NB) * CHUNK
                        pre = nc.gpsimd.drain()
                        if prev_pool is not None:
                            add_dep_helper(pre.ins, prev_pool.ins, sync=False,
                                           reason="pool order")
                        if k == 0:
                            add_dep_helper(pre.ins, tbl_touch.ins, sync=True,
                                           reason="tables")
                            add_dep_helper(pre.ins, zed.ins, sync=True,
                                           reason="idx zeroed")
                        if cidx == 0:
                            pbl = nc.gpsimd.isa(
                                Op.NEURON_ISA_TPB_OPCODE_POOL_BUFFER_LOAD,
                                {
                                    "src_mem_pattern": {
                                        "start_addr": {
                                            "addr_immediate": int(tcp_off)
                                            + img * TBL * 2
                                        },
                                        "num_elem": [TBL, 1, 1, 1],
                                        "step_elem": [1, 0, 0, 0],
                                    },
                                    "in_dtype": F16,
                                    "num_active_channels": P,
                                    "start_index": 0,
                                    "mask": TBL - 1,
                                },
                            )
                            add_dep_helper(pbl.ins, pre.ins, sync=False,
                                           reason="pool order")
                            gdep = pbl
                        else:
                            gdep = pre
                        gt = nc.gpsimd.isa(
                            Op.NEURON_ISA_TPB_OPCODE_GATHER,
                            {
                                "src_mem_pattern": {
                                    "start_addr": {
                                        "addr_immediate": int(ub_off) + so * 2
                                    },
                                    "num_elem": [CHUNK, 1, 1, 1],
                                    "step_elem": [1, 0, 0, 0],
                                },
                                "dst_mem_pattern": {
                                    "start_addr": {
                                        "addr_immediate": int(ob_off) + so * 2
                                    },
                                    "num_elem": [CHUNK, 1, 1, 1],
                                    "step_elem": [1, 0, 0, 0],
                                },
                                "in_dtype": U16,
                                "out_dtype": F16,
                                "num_active_channels": P,
                                "index_miss_behavior": 0,
                                "immediate": {"imm_bitvec_uint32": 0},
                                "free_pool_buffer": 0,
                            },
                        )
                        add_dep_helper(gt.ins, gdep.ins, sync=False,
                                       reason="pool order")
                        prev_pool = gt
                fin = nc.gpsimd.drain()
                add_dep_helper(fin.ins, prev_pool.ins, sync=False,
                               reason="pool order")
            nc.finalize()
            return nc
        with loop_cm:
            # table load + a DVE copy so pool's wait collapses onto the DVE clock
            for img in range(IMGS):
                nc.sync.dma_start(tbl_all[:, img * TBL : (img + 1) * TBL], tb_d[img])
            tbl_touch = nc.vector.tensor_copy(tbl_cp[:], tbl_all[:])

            fences = {}  # k -> drain emitted just after gather k-1 (pool order)
            outs = {}  # k -> output DMA instruction for chunk k
            pend = None  # (k, img, f0, slot) awaiting its post-gather fence
            prev_pool = None
            k = 0

            def _emit_out(p, fence):
                d = nc.scalar.dma_start(
                    os_r[p["img"], :, p["f0"] : p["f0"] + CHUNK],
                    ob[:, p["slot"] * CHUNK : (p["slot"] + 1) * CHUNK],
                )
                add_dep_helper(d.ins, fence.ins, sync=True, reason="gather done")
                outs[p["k"]] = d

            for img in range(IMGS):
                for cidx in range(NCHUNK):
                    slot = k % NB
                    f0 = cidx * CHUNK
                    so = slot * CHUNK
                    x_t = xb[:, so : so + CHUNK]
                    u_t = ub[:, so : so + CHUNK]

                    nc.sync.dma_start(x_t, xs_r[img, :, f0 : f0 + CHUNK])

                    # idx = u16(S*x - 0.5): round-nearest fp32->u16 == floor(S*x)
                    ts_u = nc.vector.tensor_scalar(
                        u_t, x_t, float(S), 0.5, ALU.mult, ALU.subtract
                    )
                    if k >= NB:
                        # gather k-NB read this ub slot; its fence is fences[k-NB+1]
                        add_dep_helper(
                            ts_u.ins, fences[k - NB + 1].ins, sync=True,
                            reason="u WAR",
                        )

                    # pool: single drain per chunk — serves as the previous
                    # gather's completion fence AND this gather's input wait
                    pre = nc.gpsimd.drain()
                    fences[k] = pre
                    if prev_pool is not None:
                        add_dep_helper(
                            pre.ins, prev_pool.ins, sync=False, reason="pool order"
                        )
                    add_dep_helper(pre.ins, ts_u.ins, sync=True, reason="u ready")
                    if k >= NB:
                        # out-DMA k-NB still reads this ob slot
                        add_dep_helper(
                            pre.ins, outs[k - NB].ins, sync=True, reason="o WAR"
                        )
                    if cidx == 0:
                        if img == 0:
                            add_dep_helper(
                                pre.ins, tbl_touch.ins, sync=True, reason="tables"
                            )
                        pbl = nc.gpsimd.isa(
                            Op.NEURON_ISA_TPB_OPCODE_POOL_BUFFER_LOAD,
                            {
                                "src_mem_pattern": {
                                    "start_addr": {
                                        "addr_immediate": int(tcp_off) + img * TBL * 2
                                    },
                                    "num_elem": [TBL, 1, 1, 1],
                                    "step_elem": [1, 0, 0, 0],
                                },
                                "in_dtype": F16,
                                "num_active_channels": P,
                                "start_index": 0,
                                "mask": TBL - 1,
                            },
                        )
                        add_dep_helper(pbl.ins, pre.ins, sync=False, reason="pool order")
                        gdep = pbl
                    else:
                        gdep = pre
                    gt = nc.gpsimd.isa(
                        Op.NEURON_ISA_TPB_OPCODE_GATHER,
                        {
                            "src_mem_pattern": {
                                "start_addr": {"addr_immediate": int(ub_off) + so * 2},
                                "num_elem": [CHUNK, 1, 1, 1],
                                "step_elem": [1, 0, 0, 0],
                            },
                            "dst_mem_pattern": {
                                "start_addr": {"addr_immediate": int(ob_off) + so * 2},
                                "num_elem": [CHUNK, 1, 1, 1],
                                "step_elem": [1, 0, 0, 0],
                            },
                            "in_dtype": U16,
                            "out_dtype": F16,
                            "num_active_channels": P,
                            "index_miss_behavior": 0,
                            "immediate": {"imm_bitvec_uint32": 0},
                            "free_pool_buffer": 0,
                        },
                    )
                    add_dep_helper(gt.ins, gdep.ins, sync=False, reason="pool order")

                    # the drain just emitted fences the PREVIOUS gather; its
                    # output can ship now
                    if pend is not None:
                        _emit_out(pend, pre)
                    pend = dict(k=k, img=img, f0=f0, slot=slot)
                    prev_pool = gt
                    k += 1
            fin = nc.gpsimd.drain()
            add_dep_helper(fin.ins, prev_pool.ins, sync=False, reason="pool order")
            _emit_out(pend, fin)

    nc.finalize()
    return nc


def _tables(un_normalized_y: np.ndarray) -> np.ndarray:
    """[B, C, TBL] fp16: dense midpoint-sampled LUT of the normalized curve."""
    u = un_normalized_y.astype(np.float64)
    h = np.logaddexp(0.0, u)  # softplus
    y = np.cumsum(h, axis=2)
    y0 = y[:, :, :1]
    yn = y[:, :, -1:]
    y = (y - y0) / (yn - y0)  # [B, C, K+1], y[0]=0, y[K]=1

    t = np.minimum((np.arange(TBL, dtype=np.float64) + 0.5) / S, 1.0)  # midpoints
    scaled = t * K
    idx0 = np.clip(np.floor(scaled), 0, K - 1).astype(np.int64)
    alpha = scaled - idx0
    y_lo = y[:, :, idx0]  # [B, C, TBL]
    y_hi = y[:, :, idx0 + 1]
    val = y_lo + alpha * (y_hi - y_lo)
    return val.astype(np.float16)


def _in_maps(x: np.ndarray, uy: np.ndarray):
    pk = _tables(uy)
    x16 = x.astype(np.float16)
    in_maps = []
    for c in range(NCORES):
        xs = x16[c * BPC : (c + 1) * BPC].reshape(IMGS, H, W)
        tb = np.ascontiguousarray(
            np.broadcast_to(
                pk[c * BPC : (c + 1) * BPC].reshape(IMGS, 1, TBL), (IMGS, P, TBL)
            )
        )
        in_maps.append({"xs": np.ascontiguousarray(xs), "tb": tb})
    return in_maps


def kernel(x: np.ndarray, un_normalized_y: np.ndarray) -> np.ndarray:
    from concourse import bass_utils

    x = np.asarray(x, dtype=np.float32)
    uy = np.asarray(un_normalized_y, dtype=np.float32)

    if "nc" not in _cached:
        _cached["nc"] = _build()
    nc = _cached["nc"]

    res = bass_utils.run_bass_kernel_spmd(
        nc, _in_maps(x, uy), core_ids=list(range(NCORES))
    )
    out = np.empty((B, C, H, W), dtype=np.float32)
    for c in range(NCORES):
        out[c * BPC : (c + 1) * BPC] = (
            res.results[c]["os"].astype(np.float32).reshape(BPC, C, H, W)
        )
    return out


# revision 8
# speedup vs baseline: 2.4826x; 1.3168x over previous
"""Per-image piecewise-linear LUT (histogram binning) kernel for Trainium2.

Strategy (pure data-parallel over 8 NeuronCores, batch sharded 2 per core):
- Host precomputes, per (b, c), a dense 512-entry nearest-neighbor table
  sampling the normalized curve at bin midpoints: tbl[j] = y((j+0.5)/S),
  S = 511.5.  With 512 bins the midpoint-sampling error is ~1e-3 norm-rel,
  far inside the 2e-2 gate, and it removes the on-device interpolation
  entirely.
- x ships as fp16 (halves input HBM traffic; fp16 quantization of x only
  perturbs the bin index by <0.3 bins).  Output is written fp16 and
  upcast on host.
- On-device per core: 6 images of [128 partitions x 8192 fp16].  Per image:
    u16 idx = u16(x * 511.5 - 0.5)   (one DVE tensor_scalar, 4x mode)
    out     = pooltable[idx]          (pool-engine PoolBufferLoad+Gather,
                                       512-entry per-channel table)
    DMA out (fp16)
- The raw Gather/PoolBufferLoad ISA instructions cannot carry semaphores
  (walrus rejects sync on unknown structs); drains bracket them and all
  cross-engine syncs land on the drains / are wired manually.
"""

import sys

sys.path.insert(0, "/opt/trn_rl_repo")

import numpy as np

B, C, H, W = 16, 3, 1024, 1024
K = 64
NCORES = 8
BPC = B // NCORES  # batches per core
IMGS = BPC * C  # images per core
P = 128
FREE = H * W // P  # 8192
CHUNK = 8192
NCHUNK = FREE // CHUNK
TBL = 512  # pool buffer entries (hardware max 512)
S = 511.5  # index scale: u = round_nearest(x*S - 0.5) in [0, 511] for x in [0,1]
NB = 3  # buffer depth

_cached = {}


def _build(loop_n=None, mode="full"):
    import contextlib
    import concourse.mybir as mybir
    from concourse.bacc import Bacc
    from concourse.tile import TileContext
    from concourse.tile_rust import add_dep_helper
    import concourse.bass_interp as _bi

    # Tile's scheduling simulator doesn't know these opcodes; no-op them there.
    _orig_visit = _bi._visit_InstISA

    def _patched_visit(isa, instruction, core_sim):
        if instruction.isa_opcode in (
            isa.Opcode.NEURON_ISA_TPB_OPCODE_POOL_BUFFER_LOAD.value,
            isa.Opcode.NEURON_ISA_TPB_OPCODE_GATHER.value,
        ):
            return
        return _orig_visit(isa, instruction, core_sim)

    _bi._visit_InstISA = _patched_visit

    nc = Bacc()
    dt = nc.isa.get_enum("NEURON_ISA_TPB_DTYPE")
    Op = nc.isa.Opcode
    ALU = mybir.AluOpType

    xs_d = nc.dram_tensor("xs", [IMGS, H, W], mybir.dt.float16, kind="ExternalInput")
    tb_d = nc.dram_tensor("tb", [IMGS, P, TBL], mybir.dt.float16, kind="ExternalInput")
    os_d = nc.dram_tensor("os", [IMGS, H, W], mybir.dt.float16, kind="ExternalOutput")

    xs_r = xs_d[:].rearrange("i (p r) c -> i p (r c)", p=P)
    os_r = os_d[:].rearrange("i (p r) c -> i p (r c)", p=P)

    with (
        nc.sbuf_tensor("tbl_all", [P, IMGS * TBL], mybir.dt.float16) as tbl_all,
        nc.sbuf_tensor("tbl_cp", [P, IMGS * TBL], mybir.dt.float16) as tbl_cp,
        nc.sbuf_tensor("xb", [P, NB * CHUNK], mybir.dt.float16) as xb,
        nc.sbuf_tensor("ub", [P, NB * CHUNK], mybir.dt.uint16) as ub,
        nc.sbuf_tensor("ob", [P, NB * CHUNK], mybir.dt.float16) as ob,
        TileContext(nc) as tc,
    ):
        ub_off, _ = nc.gpsimd._ap_to_byte_offset(ub[:])
        ob_off, _ = nc.gpsimd._ap_to_byte_offset(ob[:])
        tcp_off, _ = nc.gpsimd._ap_to_byte_offset(tbl_cp[:])
        U16 = dt.NEURON_ISA_TPB_DTYPE_UINT16.value
        F16 = dt.NEURON_ISA_TPB_DTYPE_FP16.value

        loop_cm = (
            tc.For_i(0, loop_n, 1) if loop_n is not None else contextlib.nullcontext()
        )
        if mode == "dma":
            with loop_cm:
                for img in range(IMGS):
                    for cidx in range(NCHUNK):
                        k = img * NCHUNK + cidx
                        slot = k % NB
                        f0 = cidx * CHUNK
                        so = slot * CHUNK
                        nc.sync.dma_start(
                            xb[:, so : so + CHUNK], xs_r[img, :, f0 : f0 + CHUNK]
                        )
                        nc.scalar.dma_start(
                            os_r[img, :, f0 : f0 + CHUNK], ob[:, so : so + CHUNK]
                        )
        elif mode == "pool":
            with loop_cm:
                for img in range(IMGS):
                    nc.sync.dma_start(
                        tbl_all[:, img * TBL : (img + 1) * TBL], tb_d[img]
                    )
                tbl_touch = nc.vector.tensor_copy(tbl_cp[:], tbl_all[:])
                zed = nc.vector.memset(ub[:], 0)
                prev_pool = None
                for img in range(IMGS):
                    for cidx in range(NCHUNK):
                        k = img * NCHUNK + cidx
                        so = (k % NB) * CHUNK
                        pre = nc.gpsimd.drain()
                        if prev_pool is not None:
                            add_dep_helper(pre.ins, prev_pool.ins, sync=False,
                                           reason="pool order")
                        if k == 0:
                            add_dep_helper(pre.ins, tbl_touch.ins, sync=True,
                                           reason="tables")
                            add_dep_helper(pre.ins, zed.ins, sync=True,
                                           reason="idx zeroed")
                        if cidx == 0:
                            pbl = nc.gpsimd.isa(
                                Op.NEURON_ISA_TPB_OPCODE_POOL_BUFFER_LOAD,
                                {
                                    "src_mem_pattern": {
                                        "start_addr": {
                                            "addr_immediate": int(tcp_off)
                                            + img * TBL * 2
                                        },
                                        "num_elem": [TBL, 1, 1, 1],
                                        "step_elem": [1, 0, 0, 0],
                                    },
                                    "in_dtype": F16,
                                    "num_active_channels": P,
                                    "start_index": 0,
                                    "mask": TBL - 1,
                                },
                            )
                            add_dep_helper(pbl.ins, pre.ins, sync=False,
                                           reason="pool order")
                            gdep = pbl
                        else:
                            gdep = pre
                        gt = nc.gpsimd.isa(
                            Op.NEURON_ISA_TPB_OPCODE_GATHER,
                            {
                                "src_mem_pattern": {
                                    "start_addr": {
                                        "addr_immediate": int(ub_off) + so * 2
                                    },
                                    "num_elem": [CHUNK, 1, 1, 1],
                                    "step_elem": [1, 0, 0, 0],
                                },
                                "dst_mem_pattern": {
                                    "start_addr": {
                                        "addr_immediate": int(ob_off) + so * 2
                                    },
                                    "num_elem": [CHUNK, 1, 1, 1],
                                    "step_elem": [1, 0, 0, 0],
                                },
                                "in_dtype": U16,
                                "out_dtype": F16,
                                "num_active_channels": P,
                                "index_miss_behavior": 0,
                                "immediate": {"imm_bitvec_uint32": 0},
                                "free_pool_buffer": 0,
                            },
                        )
                        add_dep_helper(gt.ins, gdep.ins, sync=False,
                                       reason="pool order")
                        prev_pool = gt
                fin = nc.gpsimd.drain()
                add_dep_helper(fin.ins, prev_pool.ins, sync=False,
                               reason="pool order")
        if mode in ("dma", "pool"):
            pass
        else:
          with loop_cm:
            # table load + a DVE copy so pool's wait collapses onto the DVE clock
            for img in range(IMGS):
                nc.sync.dma_start(tbl_all[:, img * TBL : (img + 1) * TBL], tb_d[img])
            tbl_touch = nc.vector.tensor_copy(tbl_cp[:], tbl_all[:])

            fences = {}  # k -> drain emitted just after gather k-1 (pool order)
            outs = {}  # k -> output DMA instruction for chunk k
            pend = None  # (k, img, f0, slot) awaiting its post-gather fence
            prev_pool = None
            k = 0

            def _emit_out(p, fence):
                d = nc.scalar.dma_start(
                    os_r[p["img"], :, p["f0"] : p["f0"] + CHUNK],
                    ob[:, p["slot"] * CHUNK : (p["slot"] + 1) * CHUNK],
                )
                add_dep_helper(d.ins, fence.ins, sync=True, reason="gather done")
                outs[p["k"]] = d

            for img in range(IMGS):
                for cidx in range(NCHUNK):
                    slot = k % NB
                    f0 = cidx * CHUNK
                    so = slot * CHUNK
                    x_t = xb[:, so : so + CHUNK]
                    u_t = ub[:, so : so + CHUNK]

                    nc.sync.dma_start(x_t, xs_r[img, :, f0 : f0 + CHUNK])

                    # idx = u16(S*x - 0.5): round-nearest fp32->u16 == floor(S*x)
                    ts_u = nc.vector.tensor_scalar(
                        u_t, x_t, float(S), 0.5, ALU.mult, ALU.subtract
                    )
                    if k >= NB:
                        # gather k-NB read this ub slot; its fence is fences[k-NB+1]
                        add_dep_helper(
                            ts_u.ins, fences[k - NB + 1].ins, sync=True,
                            reason="u WAR",
                        )

                    # pool: single drain per chunk — serves as the previous
                    # gather's completion fence AND this gather's input wait
                    pre = nc.gpsimd.drain()
                    fences[k] = pre
                    if prev_pool is not None:
                        add_dep_helper(
                            pre.ins, prev_pool.ins, sync=False, reason="pool order"
                        )
                    add_dep_helper(pre.ins, ts_u.ins, sync=True, reason="u ready")
                    if k >= NB:
                        # out-DMA k-NB still reads this ob slot
                        add_dep_helper(
                            pre.ins, outs[k - NB].ins, sync=True, reason="o WAR"
                        )
                    if cidx == 0:
                        if img == 0:
                            add_dep_helper(
                                pre.ins, tbl_touch.ins, sync=True, reason="tables"
                            )
                        pbl = nc.gpsimd.isa(
                            Op.NEURON_ISA_TPB_OPCODE_POOL_BUFFER_LOAD,
                            {
                                "src_mem_pattern": {
                                    "start_addr": {
                                        "addr_immediate": int(tcp_off) + img * TBL * 2
                                    },
                                    "num_elem": [TBL, 1, 1, 1],
                                    "step_elem": [1, 0, 0, 0],
                                },
                                "in_dtype": F16,
                                "num_active_channels": P,
                                "start_index": 0,
                                "mask": TBL - 1,
                            },
                        )
                        add_dep_helper(pbl.ins, pre.ins, sync=False, reason="pool order")
                        gdep = pbl
                    else:
                        gdep = pre
                    gt = nc.gpsimd.isa(
                        Op.NEURON_ISA_TPB_OPCODE_GATHER,
                        {
                            "src_mem_pattern": {
                                "start_addr": {"addr_immediate": int(ub_off) + so * 2},
                                "num_elem": [CHUNK, 1, 1, 1],
                                "step_elem": [1, 0, 0, 0],
                            },
                            "dst_mem_pattern": {
                                "start_addr": {"addr_immediate": int(ob_off) + so * 2},
                                "num_elem": [CHUNK, 1, 1, 1],
                                "step_elem": [1, 0, 0, 0],
                            },
                            "in_dtype": U16,
                            "out_dtype": F16,
                            "num_active_channels": P,
                            "index_miss_behavior": 0,
                            "immediate": {"imm_bitvec_uint32": 0},
                            "free_pool_buffer": 0,
                        },
                    )
                    add_dep_helper(gt.ins, gdep.ins, sync=False, reason="pool order")

                    # the drain just emitted fences the PREVIOUS gather; its
                    # output can ship now
                    if pend is not None:
                        _emit_out(pend, pre)
                    pend = dict(k=k, img=img, f0=f0, slot=slot)
                    prev_pool = gt
                    k += 1
            fin = nc.gpsimd.drain()
            add_dep_helper(fin.ins, prev_pool.ins, sync=False, reason="pool order")
            _emit_out(pend, fin)

    nc.finalize()
    return nc


def _tables(un_normalized_y: np.ndarray) -> np.ndarray:
    """[B, C, TBL] fp16: dense midpoint-sampled LUT of the normalized curve."""
    u = un_normalized_y.astype(np.float64)
    h = np.logaddexp(0.0, u)  # softplus
    y = np.cumsum(h, axis=2)
    y0 = y[:, :, :1]
    yn = y[:, :, -1:]
    y = (y - y0) / (yn - y0)  # [B, C, K+1], y[0]=0, y[K]=1

    t = np.minimum((np.arange(TBL, dtype=np.float64) + 0.5) / S, 1.0)  # midpoints
    scaled = t * K
    idx0 = np.clip(np.floor(scaled), 0, K - 1).astype(np.int64)
    alpha = scaled - idx0
    y_lo = y[:, :, idx0]  # [B, C, TBL]
    y_hi = y[:, :, idx0 + 1]
    val = y_lo + alpha * (y_hi - y_lo)
    return val.astype(np.float16)


def _in_maps(x: np.ndarray, uy: np.ndarray):
    pk = _tables(uy)
    x16 = x.astype(np.float16)
    in_maps = []
    for c in range(NCORES):
        xs = x16[c * BPC : (c + 1) * BPC].reshape(IMGS, H, W)
        tb = np.ascontiguousarray(
            np.broadcast_to(
                pk[c * BPC : (c + 1) * BPC].reshape(IMGS, 1, TBL), (IMGS, P, TBL)
            )
        )
        in_maps.append({"xs": np.ascontiguousarray(xs), "tb": tb})
    return in_maps


def kernel(x: np.ndarray, un_normalized_y: np.ndarray) -> np.ndarray:
    from concourse import bass_utils

    x = np.asarray(x, dtype=np.float32)
    uy = np.asarray(un_normalized_y, dtype=np.float32)

    if "nc" not in _cached:
        _cached["nc"] = _build()
    nc = _cached["nc"]

    res = bass_utils.run_bass_kernel_spmd(
        nc, _in_maps(x, uy), core_ids=list(range(NCORES))
    )
    out = np.empty((B, C, H, W), dtype=np.float32)
    for c in range(NCORES):
        out[c * BPC : (c + 1) * BPC] = (
            res.results[c]["os"].astype(np.float32).reshape(BPC, C, H, W)
        )
    return out
